# revision 27
# baseline (speedup 1.0000x reference)
# MoE routing kernel for Trainium2 (8 NeuronCores, SPMD token-parallel).
#
# Sharding: token-parallel. Each core owns 1024 of the 8192 tokens and all 8
# experts. Routing + top-2 gather/combine happen on-device per core; there is
# no cross-core communication (lb_loss partial sums are combined on host as
# part of unsharding).
#
# Numerics: LayerNorm + router run in fp32 (top-2 expert selection must match
# the fp32 reference exactly; the smallest top2/top3 logit gap in the workload
# regime is ~3e-5, so bf16 routing would mis-route tokens). Expert FFN matmuls
# run in bf16 with fp32 PSUM accumulation.

import numpy as np
import ml_dtypes
from contextlib import ExitStack

import concourse.bass as bass
import concourse.bacc as bacc
import concourse.tile as tile
from concourse import mybir
from concourse.masks import make_identity
from concourse.bass_utils import run_bass_kernel_spmd

F32 = mybir.dt.float32
BF16 = mybir.dt.bfloat16
I32 = mybir.dt.int32
AF = mybir.ActivationFunctionType
OP = mybir.AluOpType
AX = mybir.AxisListType

NCORES = 8
T = 1024          # tokens per core
D = 1024          # model dim
H = 2048          # expert hidden dim
E = 8             # experts
CAP = 320         # per-(core, expert) token capacity (max observed 284)
NSLOT = E * CAP   # 2560 compact slots per core
EPS = 1e-5
TBLK = T // 128   # 8 token blocks
DBLK = D // 128   # 8 d blocks
JBLK = H // 128   # 16 hidden blocks
SBLK = 20         # NSLOT // 128 slot blocks
LB_WEIGHT = 0.01


def _trace(nc, tc, ctx, gelu_mode="hw"):
    x = nc.dram_tensor("x", [T, D], F32, kind="ExternalInput")
    w1t = nc.dram_tensor("w1t", [E, D, H], BF16, kind="ExternalInput")
    w2t = nc.dram_tensor("w2t", [E, H, D], BF16, kind="ExternalInput")
    gwt = nc.dram_tensor("gwt", [D, E], F32, kind="ExternalInput")
    gamma = nc.dram_tensor("gamma", [1, D], F32, kind="ExternalInput")
    beta = nc.dram_tensor("beta", [1, D], F32, kind="ExternalInput")
    out = nc.dram_tensor("out", [T, D], F32, kind="ExternalOutput")
    probsum = nc.dram_tensor("probsum", [E, 1], F32, kind="ExternalOutput")

    x_re = x[:, :].rearrange("(tb p) d -> p tb d", p=128)
    out_re = out[:, :].rearrange("(tb p) d -> p tb d", p=128)

    # ---- pools ----
    consts = ctx.enter_context(tc.tile_pool(name="consts", bufs=1))
    small = ctx.enter_context(tc.tile_pool(name="small", bufs=1))
    dram = ctx.enter_context(tc.tile_pool(name="dram", bufs=1, space="DRAM"))
    ps_prob = ctx.enter_context(tc.tile_pool(name="ps_prob", bufs=1, space="PSUM"))

    # DRAM scratch
    xnbf_d = dram.tile([T, D], BF16)
    slots_d = dram.tile([NSLOT, 1], I32)
    ybuf_d = dram.tile([NSLOT, D], BF16)
    slots_dv = slots_d[:, :].rearrange("(b p) o -> p (b o)", p=128)

    # ---- constants ----
    ident = consts.tile([128, 128], F32)
    make_identity(nc, ident)
    tri = consts.tile([128, 128], F32)
    nc.vector.memset(tri, 1.0)
    # tri[u, t] = 1.0 iff u <= t  (iota = t - u >= 0)
    nc.gpsimd.affine_select(
        out=tri, in_=tri, compare_op=OP.is_ge, fill=0.0,
        base=0, pattern=[[1, 128]], channel_multiplier=-1,
    )
    onesm = consts.tile([128, 128], F32)
    nc.vector.memset(onesm, 1.0)
    ones_col = consts.tile([128, 1], F32)
    nc.vector.memset(ones_col, 1.0)
    eps_t = consts.tile([128, 1], F32)
    nc.vector.memset(eps_t, EPS)
    iota_e_i = consts.tile([128, E], I32)
    nc.gpsimd.iota(iota_e_i, pattern=[[1, E]], base=0, channel_multiplier=0)
    iota_e = consts.tile([128, E], F32)
    nc.vector.tensor_copy(out=iota_e, in_=iota_e_i)
    zer = consts.tile([128, SBLK], I32)
    nc.vector.memset(zer, 0)
    nc.sync.dma_start(out=slots_dv, in_=zer)
    ident_bf = consts.tile([128, 128], BF16)
    make_identity(nc, ident_bf)
    gwt_sb = consts.tile([128, DBLK, E], F32)
    nc.sync.dma_start(out=gwt_sb, in_=gwt[:, :].rearrange("(db p) e -> p db e", p=128))
    gamma_sb = consts.tile([128, D], F32)
    nc.sync.dma_start(out=gamma_sb, in_=gamma[:, :].to_broadcast([128, D]))
    beta_sb = consts.tile([128, D], F32)
    nc.sync.dma_start(out=beta_sb, in_=beta[:, :].to_broadcast([128, D]))

    # ---- persistent small per-token state ----
    g1_sb = small.tile([128, TBLK], F32)
    g2_sb = small.tile([128, TBLK], F32)
    flats = small.tile([128, TBLK, 2], I32)
    m_sb = small.tile([128, TBLK, E], F32)
    oh1_sb = small.tile([128, TBLK, E], F32)
    oh2_sb = small.tile([128, TBLK, E], F32)
    e1_sb = small.tile([128, TBLK], F32)
    e2_sb = small.tile([128, TBLK], F32)
    slots_sb = small.tile([128, SBLK], I32)
    mean_sb = small.tile([128, TBLK], F32)
    rstd_sb = small.tile([128, TBLK], F32)

    # ================= Phase A: LayerNorm + raw-x router =================
    # Router identity: logits = rstd*(x @ (gamma*gw)^T - mean*s) + beta@gw^T
    # where s_e = sum_d gamma_d gw[e,d]. This decouples the router matmul
    # (which needs x transposed) from the LayerNorm output path.
    ln = ExitStack()
    with ln:
        lnp = ln.enter_context(tc.tile_pool(name="ln", bufs=1))
        ps_tr = ln.enter_context(tc.tile_pool(name="ps_tr", bufs=2, space="PSUM"))
        ps_log = ln.enter_context(tc.tile_pool(name="ps_log", bufs=2, space="PSUM"))
        ps_cum = ln.enter_context(tc.tile_pool(name="ps_cum", bufs=2, space="PSUM"))
        rtmp = ln.enter_context(tc.tile_pool(name="rtmp", bufs=3))

        # b_e = beta @ gw[e]  (uses unscaled gwt)
        gamma_d = consts.tile([128, DBLK], F32)
        nc.sync.dma_start(
            out=gamma_d, in_=gamma[0, :].rearrange("(db p) -> p db", p=128))
        beta_d = consts.tile([128, DBLK], F32)
        nc.sync.dma_start(
            out=beta_d, in_=beta[0, :].rearrange("(db p) -> p db", p=128))
        psb = ps_log.tile([E, 1], F32, tag="psb", bufs=1)
        for db in range(DBLK):
            nc.tensor.matmul(out=psb, lhsT=gwt_sb[:, db, :],
                             rhs=beta_d[:, db:db + 1],
                             start=(db == 0), stop=(db == DBLK - 1))
        b_col = small.tile([E, 1], F32)
        nc.vector.tensor_copy(out=b_col, in_=psb)
        # scale gwt by gamma (d-major); afterwards gwt_sb == gamma * gw^T
        for db in range(DBLK):
            nc.vector.tensor_scalar_mul(
                out=gwt_sb[:, db, :], in0=gwt_sb[:, db, :],
                scalar1=gamma_d[:, db:db + 1])
        pss = ps_log.tile([E, 1], F32, tag="psb", bufs=1)
        for db in range(DBLK):
            nc.tensor.matmul(out=pss, lhsT=gwt_sb[:, db, :], rhs=ones_col,
                             start=(db == 0), stop=(db == DBLK - 1))
        s_col = small.tile([E, 1], F32)
        nc.vector.tensor_copy(out=s_col, in_=pss)
        # bounce through DRAM to turn the [E,1] partition vectors into
        # partition-broadcast [128,E] rows
        sb_row_d = dram.tile([2, E], F32)
        nc.sync.dma_start(out=sb_row_d[0:1, :], in_=s_col[:, :])
        nc.sync.dma_start(out=sb_row_d[1:2, :], in_=b_col[:, :])
        s_bc = consts.tile([128, E], F32)
        nc.sync.dma_start(out=s_bc, in_=sb_row_d[0:1, :].to_broadcast([128, E]))
        b_bc = consts.tile([128, E], F32)
        nc.sync.dma_start(out=b_bc, in_=sb_row_d[1:2, :].to_broadcast([128, E]))

        x_sb = lnp.tile([128, TBLK, D], F32)
        xnbf_sb = lnp.tile([128, TBLK, D], BF16)
        xT_sb = lnp.tile([128, DBLK, T], F32)
        logits_sb = lnp.tile([128, TBLK, E], F32)
        psp = ps_prob.tile([E, 1], F32)
        for tb in range(TBLK):
            nc.sync.dma_start(out=x_sb[:, tb, :], in_=x_re[:, tb, :])
            # LN stats
            stats = rtmp.tile([128, 2, nc.vector.BN_STATS_DIM], F32, tag="stats")
            xb = x_sb[:, tb, :].rearrange("p (s f) -> p s f", s=2)
            for s in range(2):
                nc.vector.bn_stats(out=stats[:, s, :], in_=xb[:, s, :])
            mv = rtmp.tile([128, nc.vector.BN_AGGR_DIM], F32, tag="mv")
            nc.vector.bn_aggr(out=mv, in_=stats)
            mean = mean_sb[:, tb:tb + 1]
            rstd = rstd_sb[:, tb:tb + 1]
            nc.vector.tensor_copy(out=mean, in_=mv[:, 0:1])
            nc.scalar.activation(out=rstd, in_=mv[:, 1:2], func=AF.Sqrt, bias=eps_t)
            nc.vector.reciprocal(out=rstd, in_=rstd)
            # normalized activations (bf16, for the expert FFN path)
            nc.gpsimd.tensor_scalar(
                out=xnbf_sb[:, tb, :], in0=x_sb[:, tb, :],
                scalar1=mean, scalar2=rstd, op0=OP.subtract, op1=OP.mult,
            )
            nc.gpsimd.tensor_tensor(
                out=xnbf_sb[:, tb, :], in0=xnbf_sb[:, tb, :], in1=gamma_sb,
                op=OP.mult)
            nc.gpsimd.tensor_tensor(
                out=xnbf_sb[:, tb, :], in0=xnbf_sb[:, tb, :], in1=beta_sb,
                op=OP.add)
            nc.sync.dma_start(
                out=xnbf_d[tb * 128:(tb + 1) * 128, :], in_=xnbf_sb[:, tb, :])
            # raw-x transpose (PE) for the router matmul
            for db in range(DBLK):
                pst = ps_tr.tile([128, 128], F32, tag="ptrans")
                nc.tensor.transpose(
                    out=pst, in_=x_sb[:, tb, db * 128:(db + 1) * 128],
                    identity=ident)
                nc.vector.tensor_copy(
                    out=xT_sb[:, db, tb * 128:(tb + 1) * 128], in_=pst)
            # raw router matmul + affine correction
            psl = ps_log.tile([128, E], F32, tag="plog")
            for db in range(DBLK):
                nc.tensor.matmul(
                    out=psl,
                    lhsT=xT_sb[:, db, tb * 128:(tb + 1) * 128],
                    rhs=gwt_sb[:, db, :],
                    start=(db == 0), stop=(db == DBLK - 1),
                )
            mus = rtmp.tile([128, E], F32, tag="mus")
            nc.vector.tensor_scalar_mul(out=mus, in0=s_bc, scalar1=mean)
            lg = logits_sb[:, tb, :]
            nc.vector.tensor_tensor(out=lg, in0=psl, in1=mus, op=OP.subtract)
            nc.vector.tensor_scalar_mul(out=lg, in0=lg, scalar1=rstd)
            nc.vector.tensor_tensor(out=lg, in0=lg, in1=b_bc, op=OP.add)
            vals = rtmp.tile([128, 8], F32, tag="vals")
            idxs = rtmp.tile([128, 8], mybir.dt.uint32, tag="idxs")
            nc.vector.max_with_indices(out_max=vals, out_indices=idxs, in_=lg)
            nc.vector.tensor_copy(out=e1_sb[:, tb:tb + 1], in_=idxs[:, 0:1])
            nc.vector.tensor_copy(out=e2_sb[:, tb:tb + 1], in_=idxs[:, 1:2])
            nv1 = rtmp.tile([128, 1], F32, tag="nv1")
            nc.vector.tensor_scalar(
                out=nv1, in0=vals[:, 0:1], scalar1=-1.0, scalar2=None, op0=OP.mult)
            # gates: softmax over the two selected logits
            z = rtmp.tile([128, 1], F32, tag="z")
            nc.scalar.activation(out=z, in_=vals[:, 1:2], func=AF.Exp, bias=nv1)
            den = rtmp.tile([128, 1], F32, tag="den")
            nc.vector.tensor_scalar(
                out=den, in0=z, scalar1=1.0, scalar2=None, op0=OP.add)
            g1c = g1_sb[:, tb:tb + 1]
            nc.vector.reciprocal(out=g1c, in_=den)
            nc.vector.tensor_tensor(
                out=g2_sb[:, tb:tb + 1], in0=z, in1=g1c, op=OP.mult)
            # full softmax over 8 logits for the load-balance loss
            prob = rtmp.tile([128, E], F32, tag="prob")
            sume = rtmp.tile([128, 1], F32, tag="sume")
            nc.scalar.activation(
                out=prob, in_=lg, func=AF.Exp, bias=nv1, accum_out=sume)
            rs = rtmp.tile([128, 1], F32, tag="rs")
            nc.vector.reciprocal(out=rs, in_=sume)
            nc.vector.tensor_scalar_mul(out=prob, in0=prob, scalar1=rs)
            nc.tensor.matmul(
                out=psp, lhsT=prob, rhs=ones_col,
                start=(tb == 0), stop=(tb == TBLK - 1),
            )
            # routing masks
            nc.vector.tensor_scalar(
                out=oh1_sb[:, tb, :], in0=iota_e, scalar1=e1_sb[:, tb:tb + 1],
                scalar2=None, op0=OP.is_equal)
            nc.vector.tensor_scalar(
                out=oh2_sb[:, tb, :], in0=iota_e, scalar1=e2_sb[:, tb:tb + 1],
                scalar2=None, op0=OP.is_equal)
            nc.vector.tensor_tensor(
                out=m_sb[:, tb, :], in0=oh1_sb[:, tb, :], in1=oh2_sb[:, tb, :],
                op=OP.add)
        probs_out = small.tile([E, 1], F32)
        nc.vector.tensor_copy(out=probs_out, in_=psp)
        nc.sync.dma_start(out=probsum[:, :], in_=probs_out)

        # ============ Phase C: prefix-sum -> compact slot ids ============
        cum_sb = lnp.tile([128, TBLK, E], F32)
        for tb in range(TBLK):
            psc = ps_cum.tile([128, E], F32, tag="pcum")
            nc.tensor.matmul(out=psc, lhsT=tri, rhs=m_sb[:, tb, :],
                             start=True, stop=(tb == 0))
            for tb2 in range(tb):
                nc.tensor.matmul(out=psc, lhsT=onesm, rhs=m_sb[:, tb2, :],
                                 start=False, stop=(tb2 == tb - 1))
            nc.vector.tensor_copy(out=cum_sb[:, tb, :], in_=psc)

        for tb in range(TBLK):
            for k, (oh, ecol) in enumerate(
                    ((oh1_sb, e1_sb), (oh2_sb, e2_sb))):
                tmp8 = rtmp.tile([128, E], F32, tag="tmp8")
                nc.vector.tensor_tensor(
                    out=tmp8, in0=oh[:, tb, :], in1=cum_sb[:, tb, :], op=OP.mult)
                pos = rtmp.tile([128, 1], F32, tag="pos")
                nc.vector.reduce_sum(out=pos, in_=tmp8, axis=AX.X)
                fl = rtmp.tile([128, 1], F32, tag="fl")
                nc.vector.tensor_scalar(
                    out=fl, in0=ecol[:, tb:tb + 1], scalar1=float(CAP),
                    scalar2=-1.0, op0=OP.mult, op1=OP.add)
                nc.vector.tensor_tensor(out=fl, in0=fl, in1=pos, op=OP.add)
                nc.vector.tensor_copy(out=flats[:, tb, k:k + 1], in_=fl)

        # scatter token ids into the (pre-zeroed) slot table
        for tb in range(TBLK):
            tokid = rtmp.tile([128, 1], I32, tag="tokid")
            nc.gpsimd.iota(
                tokid, pattern=[[1, 1]], base=tb * 128, channel_multiplier=1)
            for k in range(2):
                nc.gpsimd.indirect_dma_start(
                    out=slots_d[:, :],
                    out_offset=bass.IndirectOffsetOnAxis(
                        ap=flats[:, tb, k:k + 1], axis=0),
                    in_=tokid,
                    in_offset=None,
                    bounds_check=NSLOT - 1,
                    oob_is_err=False,
                )
        nc.sync.dma_start(out=slots_sb, in_=slots_dv)
    # ln pools (x_sb / xn_sb / xnT / logits) freed here

    # ============ Phase D: gather + transpose routed tokens ============
    # Gather token rows to SBUF, then PE-transpose (the PE is idle here)
    # straight into the d-major layout the FFN matmuls consume.
    ffn = ExitStack()
    gtp = ffn.enter_context(tc.tile_pool(name="gtp", bufs=1))
    gT_all = gtp.tile([128, DBLK, NSLOT], BF16)
    ps_gt = ffn.enter_context(tc.tile_pool(name="ps_gt", bufs=2, space="PSUM"))
    with tc.tile_pool(name="gth", bufs=4) as gth:
        for b in range(SBLK):
            gt = gth.tile([128, D], BF16, tag="gt")
            nc.gpsimd.indirect_dma_start(
                out=gt,
                out_offset=None,
                in_=xnbf_d[:, :],
                in_offset=bass.IndirectOffsetOnAxis(
                    ap=slots_sb[:, b:b + 1], axis=0),
            )
            for db in range(DBLK):
                pgt = ps_gt.tile([128, 128], BF16, tag="pgt")
                nc.tensor.transpose(
                    out=pgt, in_=gt[:, db * 128:(db + 1) * 128],
                    identity=ident_bf)
                nc.vector.tensor_copy(
                    out=gT_all[:, db, b * 128:(b + 1) * 128], in_=pgt)

    # ============ Phase E: expert FFNs on compact buffers ============
    wp = ffn.enter_context(tc.tile_pool(name="wp", bufs=3))
    ep = ffn.enter_context(tc.tile_pool(name="ep", bufs=2))
    ps_h = ffn.enter_context(tc.tile_pool(name="ps_h", bufs=2, space="PSUM"))
    ps_y = ffn.enter_context(tc.tile_pool(name="ps_y", bufs=2, space="PSUM"))
    for e in range(E):
        w1c = []
        for half in range(2):
            w = wp.tile([128, DBLK, 1024], BF16, tag="w1c")
            nc.sync.dma_start(
                out=w,
                in_=w1t[e, :, half * 1024:(half + 1) * 1024].rearrange(
                    "(db p) h -> p db h", p=128))
            w1c.append(w)
        act = ep.tile([128, JBLK, CAP], BF16, tag="act")
        for j in range(JBLK):
            half, jl = divmod(j, 8)
            psh = ps_h.tile([128, CAP], F32, tag="psh")
            for db in range(DBLK):
                nc.tensor.matmul(
                    out=psh,
                    lhsT=w1c[half][:, db, jl * 128:(jl + 1) * 128],
                    rhs=gT_all[:, db, e * CAP:(e + 1) * CAP],
                    start=(db == 0), stop=(db == DBLK - 1),
                )
            if gelu_mode == "hw":
                nc.scalar.activation(out=act[:, j, :], in_=psh, func=AF.Gelu)
            else:
                # tanh-approx gelu built from sim-supported primitives
                x3 = ep.tile([128, CAP], F32, tag="gelu_x3")
                nc.scalar.activation(out=x3, in_=psh, func=AF.Square)
                nc.vector.tensor_tensor(out=x3, in0=x3, in1=psh, op=OP.mult)
                nc.vector.tensor_scalar(
                    out=x3, in0=x3, scalar1=0.044715, scalar2=None, op0=OP.mult)
                nc.vector.tensor_tensor(out=x3, in0=x3, in1=psh, op=OP.add)
                th = ep.tile([128, CAP], F32, tag="gelu_th")
                nc.scalar.activation(
                    out=th, in_=x3, func=AF.Tanh, scale=0.7978845608028654)
                nc.vector.tensor_scalar(
                    out=th, in0=th, scalar1=1.0, scalar2=None, op0=OP.add)
                xh = ep.tile([128, CAP], F32, tag="gelu_xh")
                nc.vector.tensor_scalar(
                    out=xh, in0=psh, scalar1=0.5, scalar2=None, op0=OP.mult)
                nc.vector.tensor_tensor(
                    out=act[:, j, :], in0=xh, in1=th, op=OP.mult)
        w2c = []
        for half in range(2):
            w = wp.tile([128, DBLK, 1024], BF16, tag="w2c")
            nc.sync.dma_start(
                out=w,
                in_=w2t[e, half * 1024:(half + 1) * 1024, :].rearrange(
                    "(db p) i -> p db i", p=128))
            w2c.append(w)
        y_sb = ep.tile([128, 3, D], BF16, tag="y_sb")
        for sbk in range(3):
            ss = min(128, CAP - sbk * 128)
            for ic in range(2):
                psy = ps_y.tile([128, 512], F32, tag="psy")
                for jb in range(JBLK):
                    half, jl = divmod(jb, 8)
                    nc.tensor.matmul(
                        out=psy[:ss, :],
                        lhsT=act[:, jb, sbk * 128:sbk * 128 + ss],
                        rhs=w2c[half][:, jl, ic * 512:(ic + 1) * 512],
                        start=(jb == 0), stop=(jb == JBLK - 1),
                    )
                nc.vector.tensor_copy(
                    out=y_sb[:ss, sbk, ic * 512:(ic + 1) * 512], in_=psy[:ss, :])
        nc.sync.dma_start(
            out=ybuf_d[e * CAP:e * CAP + 256, :].rearrange(
                "(sb p) i -> p sb i", p=128),
            in_=y_sb[:, 0:2, :])
        nc.sync.dma_start(
            out=ybuf_d[e * CAP + 256:(e + 1) * CAP, :], in_=y_sb[0:64, 2, :])

    ffn.close()

    # ============ Phase F: combine ============
    cmb = ctx.enter_context(tc.tile_pool(name="cmb", bufs=3))
    for tb in range(TBLK):
        x2 = cmb.tile([128, D], F32, tag="x2")
        nc.sync.dma_start(out=x2, in_=x_re[:, tb, :])
        c1 = cmb.tile([128, D], BF16, tag="c1")
        nc.gpsimd.indirect_dma_start(
            out=c1, out_offset=None, in_=ybuf_d[:, :],
            in_offset=bass.IndirectOffsetOnAxis(ap=flats[:, tb, 0:1], axis=0))
        c2 = cmb.tile([128, D], BF16, tag="c2")
        nc.gpsimd.indirect_dma_start(
            out=c2, out_offset=None, in_=ybuf_d[:, :],
            in_offset=bass.IndirectOffsetOnAxis(ap=flats[:, tb, 1:2], axis=0))
        o = cmb.tile([128, D], F32, tag="o")
        nc.scalar.activation(
            out=o, in_=c1, func=AF.Copy, scale=g1_sb[:, tb:tb + 1])
        nc.vector.tensor_tensor(out=o, in0=o, in1=x2, op=OP.add)
        t2 = cmb.tile([128, D], F32, tag="t2")
        nc.scalar.activation(
            out=t2, in_=c2, func=AF.Copy, scale=g2_sb[:, tb:tb + 1])
        nc.vector.tensor_tensor(out=o, in0=o, in1=t2, op=OP.add)
        nc.sync.dma_start(out=out_re[:, tb, :], in_=o)


_NC_CACHE = None


def build_nc(gelu_mode="hw"):
    global _NC_CACHE
    if _NC_CACHE is not None and _NC_CACHE[0] == gelu_mode:
        return _NC_CACHE[1]
    nc = bacc.Bacc(None)
    with ExitStack() as ctx:
        tc = ctx.enter_context(tile.TileContext(nc))
        _trace(nc, tc, ctx, gelu_mode=gelu_mode)
    nc.compile()
    _NC_CACHE = (gelu_mode, nc)
    return nc


def make_in_maps(x, gate_w, w1, w2, ln_gamma, ln_beta):
    bf = ml_dtypes.bfloat16
    x_flat = np.ascontiguousarray(np.asarray(x, dtype=np.float32).reshape(-1, D))
    w1t = np.ascontiguousarray(
        np.asarray(w1, dtype=np.float32).transpose(0, 2, 1)).astype(bf)
    w2t = np.ascontiguousarray(
        np.asarray(w2, dtype=np.float32).transpose(0, 2, 1)).astype(bf)
    gwt = np.ascontiguousarray(np.asarray(gate_w, dtype=np.float32).T)
    gm = np.ascontiguousarray(np.asarray(ln_gamma, dtype=np.float32).reshape(1, D))
    bt = np.ascontiguousarray(np.asarray(ln_beta, dtype=np.float32).reshape(1, D))
    in_maps = []
    for c in range(NCORES):
        in_maps.append({
            "x": np.ascontiguousarray(x_flat[c * T:(c + 1) * T]),
            "w1t": w1t, "w2t": w2t, "gwt": gwt, "gamma": gm, "beta": bt,
        })
    return in_maps


def finish(results):
    out = np.concatenate([r["out"] for r in results], axis=0)
    out = out.reshape(4, 2048, D)
    psum = np.zeros(E, dtype=np.float32)
    for r in results:
        psum += r["probsum"].reshape(E)
    avg = (psum / np.float32(NCORES * T)).astype(np.float32)
    lb = (np.std(avg, ddof=1) / (np.mean(avg) + 1e-6)) ** 2
    return out, np.float32(lb * LB_WEIGHT)


def kernel(**inputs):
    nc = build_nc()
    in_maps = make_in_maps(**inputs)
    res = run_bass_kernel_spmd(nc, in_maps, core_ids=list(range(NCORES)))
    return finish(res.results)


if __name__ == "__main__":
    import jax
    key = jax.random.key(0)
    ks = jax.random.split(key, 4)
    import jax.numpy as jnp
    inputs = {
        "x": jax.random.normal(ks[0], (4, 2048, D), dtype=jnp.float32),
        "gate_w": jax.random.normal(ks[1], (E, D), dtype=jnp.float32) * 0.02,
        "w1": jax.random.normal(ks[2], (E, H, D), dtype=jnp.float32) * 0.02,
        "w2": jax.random.normal(ks[3], (E, D, H), dtype=jnp.float32) * 0.02,
        "ln_gamma": jnp.ones((D,), dtype=jnp.float32),
        "ln_beta": jnp.zeros((D,), dtype=jnp.float32),
    }
    o, lb = kernel(**{k: np.asarray(v) for k, v in inputs.items()})
    print("out", o.shape, o.dtype, "lb", lb)


# revision 29
# speedup vs baseline: 1.2659x; 1.2659x over previous
# MoE routing kernel for Trainium2 (8 NeuronCores, SPMD token-parallel).
#
# Sharding: token-parallel. Each core owns 1024 of the 8192 tokens and all 8
# experts. Routing + top-2 gather/combine happen on-device per core; there is
# no cross-core communication (lb_loss partial sums are combined on host as
# part of unsharding).
#
# Numerics: LayerNorm + router run in fp32 (top-2 expert selection must match
# the fp32 reference exactly; the smallest top2/top3 logit gap in the workload
# regime is ~3e-5, so bf16 routing would mis-route tokens). Expert FFN matmuls
# run in bf16 with fp32 PSUM accumulation.

import numpy as np
import ml_dtypes
from contextlib import ExitStack

import concourse.bass as bass
import concourse.bacc as bacc
import concourse.tile as tile
from concourse import mybir
from concourse.masks import make_identity
from concourse.bass_utils import run_bass_kernel_spmd

F32 = mybir.dt.float32
BF16 = mybir.dt.bfloat16
I32 = mybir.dt.int32
AF = mybir.ActivationFunctionType
OP = mybir.AluOpType
AX = mybir.AxisListType

NCORES = 8
T = 1024          # tokens per core
D = 1024          # model dim
H = 2048          # expert hidden dim
E = 8             # experts
CAP = 320         # per-(core, expert) token capacity (max observed 284)
NSLOT = E * CAP   # 2560 compact slots per core
EPS = 1e-5
TBLK = T // 128   # 8 token blocks
DBLK = D // 128   # 8 d blocks
JBLK = H // 128   # 16 hidden blocks
SBLK = 20         # NSLOT // 128 slot blocks
LB_WEIGHT = 0.01


def _trace(nc, tc, ctx, gelu_mode="hw"):
    x = nc.dram_tensor("x", [T, D], F32, kind="ExternalInput")
    w1t = nc.dram_tensor("w1t", [E, D, H], BF16, kind="ExternalInput")
    w2t = nc.dram_tensor("w2t", [E, H, D], BF16, kind="ExternalInput")
    gwt = nc.dram_tensor("gwt", [D, E], F32, kind="ExternalInput")
    gamma = nc.dram_tensor("gamma", [1, D], F32, kind="ExternalInput")
    beta = nc.dram_tensor("beta", [1, D], F32, kind="ExternalInput")
    out = nc.dram_tensor("out", [T, D], F32, kind="ExternalOutput")
    probsum = nc.dram_tensor("probsum", [E, 1], F32, kind="ExternalOutput")

    x_re = x[:, :].rearrange("(tb p) d -> p tb d", p=128)
    out_re = out[:, :].rearrange("(tb p) d -> p tb d", p=128)

    # ---- pools ----
    consts = ctx.enter_context(tc.tile_pool(name="consts", bufs=1))
    small = ctx.enter_context(tc.tile_pool(name="small", bufs=1))
    dram = ctx.enter_context(tc.tile_pool(name="dram", bufs=1, space="DRAM"))
    ps_prob = ctx.enter_context(tc.tile_pool(name="ps_prob", bufs=1, space="PSUM"))

    # DRAM scratch
    xnbf_d = dram.tile([T, D], BF16)
    slots_d = dram.tile([NSLOT, 1], I32)
    ybuf_d = dram.tile([NSLOT, D], BF16)
    slots_dv = slots_d[:, :].rearrange("(b p) o -> p (b o)", p=128)

    # ---- constants ----
    ident = consts.tile([128, 128], F32)
    make_identity(nc, ident)
    tri = consts.tile([128, 128], F32)
    nc.vector.memset(tri, 1.0)
    # tri[u, t] = 1.0 iff u <= t  (iota = t - u >= 0)
    nc.gpsimd.affine_select(
        out=tri, in_=tri, compare_op=OP.is_ge, fill=0.0,
        base=0, pattern=[[1, 128]], channel_multiplier=-1,
    )
    onesm = consts.tile([128, 128], F32)
    nc.vector.memset(onesm, 1.0)
    ones_col = consts.tile([128, 1], F32)
    nc.vector.memset(ones_col, 1.0)
    eps_t = consts.tile([128, 1], F32)
    nc.vector.memset(eps_t, EPS)
    iota_e_i = consts.tile([128, E], I32)
    nc.gpsimd.iota(iota_e_i, pattern=[[1, E]], base=0, channel_multiplier=0)
    iota_e = consts.tile([128, E], F32)
    nc.vector.tensor_copy(out=iota_e, in_=iota_e_i)
    zer = consts.tile([128, SBLK], I32)
    nc.vector.memset(zer, 0)
    nc.sync.dma_start(out=slots_dv, in_=zer)
    ident_bf = consts.tile([128, 128], BF16)
    make_identity(nc, ident_bf)
    gwt_sb = consts.tile([128, DBLK, E], F32)
    nc.sync.dma_start(out=gwt_sb, in_=gwt[:, :].rearrange("(db p) e -> p db e", p=128))
    gamma_sb = consts.tile([128, D], F32)
    nc.sync.dma_start(out=gamma_sb, in_=gamma[:, :].to_broadcast([128, D]))
    beta_sb = consts.tile([128, D], F32)
    nc.sync.dma_start(out=beta_sb, in_=beta[:, :].to_broadcast([128, D]))

    # ---- persistent small per-token state ----
    g1_sb = small.tile([128, TBLK], F32)
    g2_sb = small.tile([128, TBLK], F32)
    flats = small.tile([128, TBLK, 2], I32)
    m_sb = small.tile([128, TBLK, E], F32)
    oh1_sb = small.tile([128, TBLK, E], F32)
    oh2_sb = small.tile([128, TBLK, E], F32)
    e1_sb = small.tile([128, TBLK], F32)
    e2_sb = small.tile([128, TBLK], F32)
    slots_sb = small.tile([128, SBLK], I32)
    mean_sb = small.tile([128, TBLK], F32)
    rstd_sb = small.tile([128, TBLK], F32)

    # ================= Phase A: LayerNorm + raw-x router =================
    # Router identity: logits = rstd*(x @ (gamma*gw)^T - mean*s) + beta@gw^T
    # where s_e = sum_d gamma_d gw[e,d]. This decouples the router matmul
    # (which needs x transposed) from the LayerNorm output path.
    ln = ExitStack()
    with ln:
        lnp = ln.enter_context(tc.tile_pool(name="ln", bufs=1))
        ps_tr = ln.enter_context(tc.tile_pool(name="ps_tr", bufs=2, space="PSUM"))
        ps_log = ln.enter_context(tc.tile_pool(name="ps_log", bufs=2, space="PSUM"))
        ps_cum = ln.enter_context(tc.tile_pool(name="ps_cum", bufs=2, space="PSUM"))
        rtmp = ln.enter_context(tc.tile_pool(name="rtmp", bufs=3))

        # b_e = beta @ gw[e]  (uses unscaled gwt)
        gamma_d = consts.tile([128, DBLK], F32)
        nc.sync.dma_start(
            out=gamma_d, in_=gamma[0, :].rearrange("(db p) -> p db", p=128))
        beta_d = consts.tile([128, DBLK], F32)
        nc.sync.dma_start(
            out=beta_d, in_=beta[0, :].rearrange("(db p) -> p db", p=128))
        psb = ps_log.tile([E, 1], F32, tag="psb", bufs=1)
        for db in range(DBLK):
            nc.tensor.matmul(out=psb, lhsT=gwt_sb[:, db, :],
                             rhs=beta_d[:, db:db + 1],
                             start=(db == 0), stop=(db == DBLK - 1))
        b_col = small.tile([E, 1], F32)
        nc.vector.tensor_copy(out=b_col, in_=psb)
        # scale gwt by gamma (d-major); afterwards gwt_sb == gamma * gw^T
        for db in range(DBLK):
            nc.vector.tensor_scalar_mul(
                out=gwt_sb[:, db, :], in0=gwt_sb[:, db, :],
                scalar1=gamma_d[:, db:db + 1])
        pss = ps_log.tile([E, 1], F32, tag="psb", bufs=1)
        for db in range(DBLK):
            nc.tensor.matmul(out=pss, lhsT=gwt_sb[:, db, :], rhs=ones_col,
                             start=(db == 0), stop=(db == DBLK - 1))
        s_col = small.tile([E, 1], F32)
        nc.vector.tensor_copy(out=s_col, in_=pss)
        # bounce through DRAM to turn the [E,1] partition vectors into
        # partition-broadcast [128,E] rows
        sb_row_d = dram.tile([2, E], F32)
        nc.sync.dma_start(out=sb_row_d[0:1, :], in_=s_col[:, :])
        nc.sync.dma_start(out=sb_row_d[1:2, :], in_=b_col[:, :])
        s_bc = consts.tile([128, E], F32)
        nc.sync.dma_start(out=s_bc, in_=sb_row_d[0:1, :].to_broadcast([128, E]))
        b_bc = consts.tile([128, E], F32)
        nc.sync.dma_start(out=b_bc, in_=sb_row_d[1:2, :].to_broadcast([128, E]))

        x_sb = lnp.tile([128, TBLK, D], F32)
        xnbf_sb = lnp.tile([128, TBLK, D], BF16)
        xT_sb = lnp.tile([128, DBLK, T], F32)
        logits_sb = lnp.tile([128, TBLK, E], F32)
        psp = ps_prob.tile([E, 1], F32)
        for tb in range(TBLK):
            nc.sync.dma_start(out=x_sb[:, tb, :], in_=x_re[:, tb, :])
            # LN stats
            stats = rtmp.tile([128, 2, nc.vector.BN_STATS_DIM], F32, tag="stats")
            xb = x_sb[:, tb, :].rearrange("p (s f) -> p s f", s=2)
            for s in range(2):
                nc.vector.bn_stats(out=stats[:, s, :], in_=xb[:, s, :])
            mv = rtmp.tile([128, nc.vector.BN_AGGR_DIM], F32, tag="mv")
            nc.vector.bn_aggr(out=mv, in_=stats)
            mean = mean_sb[:, tb:tb + 1]
            rstd = rstd_sb[:, tb:tb + 1]
            nc.vector.tensor_copy(out=mean, in_=mv[:, 0:1])
            nc.scalar.activation(out=rstd, in_=mv[:, 1:2], func=AF.Sqrt, bias=eps_t)
            nc.vector.reciprocal(out=rstd, in_=rstd)
            # normalized activations (bf16, for the expert FFN path)
            nc.vector.tensor_scalar(
                out=xnbf_sb[:, tb, :], in0=x_sb[:, tb, :],
                scalar1=mean, scalar2=rstd, op0=OP.subtract, op1=OP.mult,
            )
            nc.vector.tensor_tensor(
                out=xnbf_sb[:, tb, :], in0=xnbf_sb[:, tb, :], in1=gamma_sb,
                op=OP.mult)
            nc.vector.tensor_tensor(
                out=xnbf_sb[:, tb, :], in0=xnbf_sb[:, tb, :], in1=beta_sb,
                op=OP.add)
            nc.sync.dma_start(
                out=xnbf_d[tb * 128:(tb + 1) * 128, :], in_=xnbf_sb[:, tb, :])
            # raw-x transpose (PE) for the router matmul
            for db in range(DBLK):
                pst = ps_tr.tile([128, 128], F32, tag="ptrans")
                nc.tensor.transpose(
                    out=pst, in_=x_sb[:, tb, db * 128:(db + 1) * 128],
                    identity=ident)
                nc.vector.tensor_copy(
                    out=xT_sb[:, db, tb * 128:(tb + 1) * 128], in_=pst)
            # raw router matmul + affine correction
            psl = ps_log.tile([128, E], F32, tag="plog")
            for db in range(DBLK):
                nc.tensor.matmul(
                    out=psl,
                    lhsT=xT_sb[:, db, tb * 128:(tb + 1) * 128],
                    rhs=gwt_sb[:, db, :],
                    start=(db == 0), stop=(db == DBLK - 1),
                )
            mus = rtmp.tile([128, E], F32, tag="mus")
            nc.vector.tensor_scalar_mul(out=mus, in0=s_bc, scalar1=mean)
            lg = logits_sb[:, tb, :]
            nc.vector.tensor_tensor(out=lg, in0=psl, in1=mus, op=OP.subtract)
            nc.vector.tensor_scalar_mul(out=lg, in0=lg, scalar1=rstd)
            nc.vector.tensor_tensor(out=lg, in0=lg, in1=b_bc, op=OP.add)
            vals = rtmp.tile([128, 8], F32, tag="vals")
            idxs = rtmp.tile([128, 8], mybir.dt.uint32, tag="idxs")
            nc.vector.max_with_indices(out_max=vals, out_indices=idxs, in_=lg)
            nc.vector.tensor_copy(out=e1_sb[:, tb:tb + 1], in_=idxs[:, 0:1])
            nc.vector.tensor_copy(out=e2_sb[:, tb:tb + 1], in_=idxs[:, 1:2])
            nv1 = rtmp.tile([128, 1], F32, tag="nv1")
            nc.vector.tensor_scalar(
                out=nv1, in0=vals[:, 0:1], scalar1=-1.0, scalar2=None, op0=OP.mult)
            # gates: softmax over the two selected logits
            z = rtmp.tile([128, 1], F32, tag="z")
            nc.scalar.activation(out=z, in_=vals[:, 1:2], func=AF.Exp, bias=nv1)
            den = rtmp.tile([128, 1], F32, tag="den")
            nc.vector.tensor_scalar(
                out=den, in0=z, scalar1=1.0, scalar2=None, op0=OP.add)
            g1c = g1_sb[:, tb:tb + 1]
            nc.vector.reciprocal(out=g1c, in_=den)
            nc.vector.tensor_tensor(
                out=g2_sb[:, tb:tb + 1], in0=z, in1=g1c, op=OP.mult)
            # full softmax over 8 logits for the load-balance loss
            prob = rtmp.tile([128, E], F32, tag="prob")
            sume = rtmp.tile([128, 1], F32, tag="sume")
            nc.scalar.activation(
                out=prob, in_=lg, func=AF.Exp, bias=nv1, accum_out=sume)
            rs = rtmp.tile([128, 1], F32, tag="rs")
            nc.vector.reciprocal(out=rs, in_=sume)
            nc.vector.tensor_scalar_mul(out=prob, in0=prob, scalar1=rs)
            nc.tensor.matmul(
                out=psp, lhsT=prob, rhs=ones_col,
                start=(tb == 0), stop=(tb == TBLK - 1),
            )
            # routing masks
            nc.vector.tensor_scalar(
                out=oh1_sb[:, tb, :], in0=iota_e, scalar1=e1_sb[:, tb:tb + 1],
                scalar2=None, op0=OP.is_equal)
            nc.vector.tensor_scalar(
                out=oh2_sb[:, tb, :], in0=iota_e, scalar1=e2_sb[:, tb:tb + 1],
                scalar2=None, op0=OP.is_equal)
            nc.vector.tensor_tensor(
                out=m_sb[:, tb, :], in0=oh1_sb[:, tb, :], in1=oh2_sb[:, tb, :],
                op=OP.add)
            # prefix-sum across token blocks (PE) and compact slot ids
            psc = ps_cum.tile([128, E], F32, tag="pcum")
            nc.tensor.matmul(out=psc, lhsT=tri, rhs=m_sb[:, tb, :],
                             start=True, stop=(tb == 0))
            for tb2 in range(tb):
                nc.tensor.matmul(out=psc, lhsT=onesm, rhs=m_sb[:, tb2, :],
                                 start=False, stop=(tb2 == tb - 1))
            cum_t = rtmp.tile([128, E], F32, tag="cum_t")
            nc.vector.tensor_copy(out=cum_t, in_=psc)
            tokid = rtmp.tile([128, 1], I32, tag="tokid")
            nc.gpsimd.iota(
                tokid, pattern=[[1, 1]], base=tb * 128, channel_multiplier=1)
            for k, (oh, ecol) in enumerate(
                    ((oh1_sb, e1_sb), (oh2_sb, e2_sb))):
                tmp8 = rtmp.tile([128, E], F32, tag="tmp8")
                nc.vector.tensor_tensor(
                    out=tmp8, in0=oh[:, tb, :], in1=cum_t, op=OP.mult)
                pos = rtmp.tile([128, 1], F32, tag="pos")
                nc.vector.reduce_sum(out=pos, in_=tmp8, axis=AX.X)
                fl = rtmp.tile([128, 1], F32, tag="fl")
                nc.vector.tensor_scalar(
                    out=fl, in0=ecol[:, tb:tb + 1], scalar1=float(CAP),
                    scalar2=-1.0, op0=OP.mult, op1=OP.add)
                nc.vector.tensor_tensor(out=fl, in0=fl, in1=pos, op=OP.add)
                nc.vector.tensor_copy(out=flats[:, tb, k:k + 1], in_=fl)
                nc.gpsimd.indirect_dma_start(
                    out=slots_d[:, :],
                    out_offset=bass.IndirectOffsetOnAxis(
                        ap=flats[:, tb, k:k + 1], axis=0),
                    in_=tokid,
                    in_offset=None,
                    bounds_check=NSLOT - 1,
                    oob_is_err=False,
                )
        probs_out = small.tile([E, 1], F32)
        nc.vector.tensor_copy(out=probs_out, in_=psp)
        nc.sync.dma_start(out=probsum[:, :], in_=probs_out)

        nc.sync.dma_start(out=slots_sb, in_=slots_dv)
    # ln pools (x_sb / xn_sb / xnT / logits) freed here

    # ============ Phase D: gather + transpose routed tokens ============
    # Gather token rows to SBUF, then PE-transpose (the PE is idle here)
    # straight into the d-major layout the FFN matmuls consume.
    ffn = ExitStack()
    gtp = ffn.enter_context(tc.tile_pool(name="gtp", bufs=1))
    gT_all = gtp.tile([128, DBLK, NSLOT], BF16)
    ps_gt = ffn.enter_context(tc.tile_pool(name="ps_gt", bufs=2, space="PSUM"))
    with tc.tile_pool(name="gth", bufs=4) as gth:
        for b in range(SBLK):
            gt = gth.tile([128, D], BF16, tag="gt")
            nc.gpsimd.indirect_dma_start(
                out=gt,
                out_offset=None,
                in_=xnbf_d[:, :],
                in_offset=bass.IndirectOffsetOnAxis(
                    ap=slots_sb[:, b:b + 1], axis=0),
            )
            for db in range(DBLK):
                pgt = ps_gt.tile([128, 128], BF16, tag="pgt")
                nc.tensor.transpose(
                    out=pgt, in_=gt[:, db * 128:(db + 1) * 128],
                    identity=ident_bf)
                nc.vector.tensor_copy(
                    out=gT_all[:, db, b * 128:(b + 1) * 128], in_=pgt)

    # ============ Phase E: expert FFNs on compact buffers ============
    wp = ffn.enter_context(tc.tile_pool(name="wp", bufs=3))
    ep = ffn.enter_context(tc.tile_pool(name="ep", bufs=2))
    ps_h = ffn.enter_context(tc.tile_pool(name="ps_h", bufs=2, space="PSUM"))
    ps_y = ffn.enter_context(tc.tile_pool(name="ps_y", bufs=2, space="PSUM"))
    for e in range(E):
        w1c = []
        for half in range(2):
            w = wp.tile([128, DBLK, 1024], BF16, tag="w1c")
            nc.sync.dma_start(
                out=w,
                in_=w1t[e, :, half * 1024:(half + 1) * 1024].rearrange(
                    "(db p) h -> p db h", p=128))
            w1c.append(w)
        act = ep.tile([128, JBLK, CAP], BF16, tag="act")
        for j in range(JBLK):
            half, jl = divmod(j, 8)
            psh = ps_h.tile([128, CAP], F32, tag="psh")
            for db in range(DBLK):
                nc.tensor.matmul(
                    out=psh,
                    lhsT=w1c[half][:, db, jl * 128:(jl + 1) * 128],
                    rhs=gT_all[:, db, e * CAP:(e + 1) * CAP],
                    start=(db == 0), stop=(db == DBLK - 1),
                )
            if gelu_mode == "hw":
                nc.scalar.activation(out=act[:, j, :], in_=psh, func=AF.Gelu)
            else:
                # tanh-approx gelu built from sim-supported primitives
                x3 = ep.tile([128, CAP], F32, tag="gelu_x3")
                nc.scalar.activation(out=x3, in_=psh, func=AF.Square)
                nc.vector.tensor_tensor(out=x3, in0=x3, in1=psh, op=OP.mult)
                nc.vector.tensor_scalar(
                    out=x3, in0=x3, scalar1=0.044715, scalar2=None, op0=OP.mult)
                nc.vector.tensor_tensor(out=x3, in0=x3, in1=psh, op=OP.add)
                th = ep.tile([128, CAP], F32, tag="gelu_th")
                nc.scalar.activation(
                    out=th, in_=x3, func=AF.Tanh, scale=0.7978845608028654)
                nc.vector.tensor_scalar(
                    out=th, in0=th, scalar1=1.0, scalar2=None, op0=OP.add)
                xh = ep.tile([128, CAP], F32, tag="gelu_xh")
                nc.vector.tensor_scalar(
                    out=xh, in0=psh, scalar1=0.5, scalar2=None, op0=OP.mult)
                nc.vector.tensor_tensor(
                    out=act[:, j, :], in0=xh, in1=th, op=OP.mult)
        w2c = []
        for half in range(2):
            w = wp.tile([128, DBLK, 1024], BF16, tag="w2c")
            nc.sync.dma_start(
                out=w,
                in_=w2t[e, half * 1024:(half + 1) * 1024, :].rearrange(
                    "(db p) i -> p db i", p=128))
            w2c.append(w)
        y_sb = ep.tile([128, 3, D], BF16, tag="y_sb")
        for sbk in range(3):
            ss = min(128, CAP - sbk * 128)
            for ic in range(2):
                psy = ps_y.tile([128, 512], F32, tag="psy")
                for jb in range(JBLK):
                    half, jl = divmod(jb, 8)
                    nc.tensor.matmul(
                        out=psy[:ss, :],
                        lhsT=act[:, jb, sbk * 128:sbk * 128 + ss],
                        rhs=w2c[half][:, jl, ic * 512:(ic + 1) * 512],
                        start=(jb == 0), stop=(jb == JBLK - 1),
                    )
                nc.vector.tensor_copy(
                    out=y_sb[:ss, sbk, ic * 512:(ic + 1) * 512], in_=psy[:ss, :])
        nc.sync.dma_start(
            out=ybuf_d[e * CAP:e * CAP + 256, :].rearrange(
                "(sb p) i -> p sb i", p=128),
            in_=y_sb[:, 0:2, :])
        nc.sync.dma_start(
            out=ybuf_d[e * CAP + 256:(e + 1) * CAP, :], in_=y_sb[0:64, 2, :])

    ffn.close()

    # ============ Phase F: combine ============
    cmb = ctx.enter_context(tc.tile_pool(name="cmb", bufs=3))
    for tb in range(TBLK):
        x2 = cmb.tile([128, D], F32, tag="x2")
        nc.sync.dma_start(out=x2, in_=x_re[:, tb, :])
        c1 = cmb.tile([128, D], BF16, tag="c1")
        nc.gpsimd.indirect_dma_start(
            out=c1, out_offset=None, in_=ybuf_d[:, :],
            in_offset=bass.IndirectOffsetOnAxis(ap=flats[:, tb, 0:1], axis=0))
        c2 = cmb.tile([128, D], BF16, tag="c2")
        nc.gpsimd.indirect_dma_start(
            out=c2, out_offset=None, in_=ybuf_d[:, :],
            in_offset=bass.IndirectOffsetOnAxis(ap=flats[:, tb, 1:2], axis=0))
        o = cmb.tile([128, D], F32, tag="o")
        nc.scalar.activation(
            out=o, in_=c1, func=AF.Copy, scale=g1_sb[:, tb:tb + 1])
        nc.vector.tensor_tensor(out=o, in0=o, in1=x2, op=OP.add)
        t2 = cmb.tile([128, D], F32, tag="t2")
        nc.scalar.activation(
            out=t2, in_=c2, func=AF.Copy, scale=g2_sb[:, tb:tb + 1])
        nc.vector.tensor_tensor(out=o, in0=o, in1=t2, op=OP.add)
        nc.sync.dma_start(out=out_re[:, tb, :], in_=o)


_NC_CACHE = None


def build_nc(gelu_mode="hw"):
    global _NC_CACHE
    if _NC_CACHE is not None and _NC_CACHE[0] == gelu_mode:
        return _NC_CACHE[1]
    nc = bacc.Bacc(None)
    with ExitStack() as ctx:
        tc = ctx.enter_context(tile.TileContext(nc))
        _trace(nc, tc, ctx, gelu_mode=gelu_mode)
    nc.compile()
    _NC_CACHE = (gelu_mode, nc)
    return nc


def make_in_maps(x, gate_w, w1, w2, ln_gamma, ln_beta):
    bf = ml_dtypes.bfloat16
    x_flat = np.ascontiguousarray(np.asarray(x, dtype=np.float32).reshape(-1, D))
    w1t = np.ascontiguousarray(
        np.asarray(w1, dtype=np.float32).transpose(0, 2, 1)).astype(bf)
    w2t = np.ascontiguousarray(
        np.asarray(w2, dtype=np.float32).transpose(0, 2, 1)).astype(bf)
    gwt = np.ascontiguousarray(np.asarray(gate_w, dtype=np.float32).T)
    gm = np.ascontiguousarray(np.asarray(ln_gamma, dtype=np.float32).reshape(1, D))
    bt = np.ascontiguousarray(np.asarray(ln_beta, dtype=np.float32).reshape(1, D))
    in_maps = []
    for c in range(NCORES):
        in_maps.append({
            "x": np.ascontiguousarray(x_flat[c * T:(c + 1) * T]),
            "w1t": w1t, "w2t": w2t, "gwt": gwt, "gamma": gm, "beta": bt,
        })
    return in_maps


def finish(results):
    out = np.concatenate([r["out"] for r in results], axis=0)
    out = out.reshape(4, 2048, D)
    psum = np.zeros(E, dtype=np.float32)
    for r in results:
        psum += r["probsum"].reshape(E)
    avg = (psum / np.float32(NCORES * T)).astype(np.float32)
    lb = (np.std(avg, ddof=1) / (np.mean(avg) + 1e-6)) ** 2
    return out, np.float32(lb * LB_WEIGHT)


def kernel(**inputs):
    nc = build_nc()
    in_maps = make_in_maps(**inputs)
    res = run_bass_kernel_spmd(nc, in_maps, core_ids=list(range(NCORES)))
    return finish(res.results)


if __name__ == "__main__":
    import jax
    key = jax.random.key(0)
    ks = jax.random.split(key, 4)
    import jax.numpy as jnp
    inputs = {
        "x": jax.random.normal(ks[0], (4, 2048, D), dtype=jnp.float32),
        "gate_w": jax.random.normal(ks[1], (E, D), dtype=jnp.float32) * 0.02,
        "w1": jax.random.normal(ks[2], (E, H, D), dtype=jnp.float32) * 0.02,
        "w2": jax.random.normal(ks[3], (E, D, H), dtype=jnp.float32) * 0.02,
        "ln_gamma": jnp.ones((D,), dtype=jnp.float32),
        "ln_beta": jnp.zeros((D,), dtype=jnp.float32),
    }
    o, lb = kernel(**{k: np.asarray(v) for k, v in inputs.items()})
    print("out", o.shape, o.dtype, "lb", lb)
